# revision 40
# baseline (speedup 1.0000x reference)
"""Trainium2 Bass kernel for nn_ActionLearner (per-sample-expert dense MLP).

reference:
    w1,b1 = fc1_table[domain_id]   # per-sample (512,1024) + (1024,)
    w2,b2 = fc2_table[domain_id]   # per-sample (1024,256) + (256,)
    out = gelu(x @ w1 + b1) @ w2 + b2          # x: (64, 256, 512)

Sharding: data-parallel over batch across 8 NeuronCores (8 samples/core).
The embedding-table gather runs on host (per-sample weights stay local to
each core's batch shard); x is host-transposed to (IN, T) per sample so
both matmuls run with zero on-device transposes:

    fc1: hT[HID,T]  = accumulate over IN of (w1 as lhsT) x (xT as rhs)
    act: gelu(hT + b1) with b1 as a per-partition bias on ACT
    fc2: oT[OUT,T]  = accumulate over HID of (w2 as lhsT) x (hT as rhs)
    out: oT DMA'd out bf16, host transposes back to (T, OUT) f32

Matmul operands are bf16 (f32 PSUM accumulation); biases stay f32.
Each sample's w1|w2|xT are host-packed partition-major into one tensor so
the whole sample loads with a single large DMA (HWDGE issue is ~0.7us per
dma_start on the SP sequencer — fewer, bigger transfers). Bias load and
output stores ride SWDGE on the otherwise-idle GpSimd engine.
"""

import numpy as np
import ml_dtypes

B, T = 64, 256
IN, HID, OUT = 512, 1024, 256
N_CORES = 8
SPC = B // N_CORES  # samples per core
KT1 = IN // 128     # fc1 contraction tiles
MT1 = HID // 128    # fc1 output-partition tiles
KT2 = HID // 128    # fc2 contraction tiles
MT2 = OUT // 128    # fc2 output-partition tiles

W1W = KT1 * HID           # 4096 bf16 words per partition
W2W = KT2 * OUT           # 2048
XTW = KT1 * T             # 1024
DATW = W1W + W2W + XTW    # 7168
XTOFF = 0                 # packed order [xt | w1 | w2]
W1OFF = XTW
W2OFF = XTW + W1W
BIASW = SPC * (MT1 + MT2)  # 80 f32 words per partition
B2COL = SPC * MT1

_CACHE = {}


def _split_multi_waits(nc):
    """This container's walrus build accepts at most ONE sync-wait per
    instruction. Hoist all but the last wait of each instruction onto fresh
    same-engine nops inserted immediately before it — identical semantics,
    engine queues execute in block order."""
    import concourse.mybir as mybir

    f = nc.m.functions[0]
    for bb in f.blocks:
        insts = bb.instructions
        if not any(
            i.sync_info and i.sync_info.on_wait and len(i.sync_info.on_wait) > 1
            for i in insts
        ):
            continue
        new_list = []
        for inst in list(insts):
            si = inst.sync_info
            if si and si.on_wait and len(si.on_wait) > 1:
                extra, keep = si.on_wait[:-1], si.on_wait[-1:]
                si.on_wait = keep
                for w in extra:
                    nop = nc.engines[inst.engine].nop(nofuse=True).ins
                    for b2 in f.blocks:
                        if b2.instructions and b2.instructions[-1] is nop:
                            b2.instructions.pop()
                            break
                    nop.sync_info = mybir.SyncInfo(on_wait=[w], on_update=[])
                    new_list.append(nop)
            new_list.append(inst)
        insts[:] = new_list


def _cheap_drain_and_barrier(self, tick_clock, wait_clock):
    """TileContext exit for a kernel where the context is the last thing in
    the program: drain + one barrier + sem clears, skipping the trailing
    all-engine barrier (nothing runs after the clears; engines just halt)."""
    from concourse.vector_clock import ScopedClock

    drain_inst = self.nc.sync.drain()
    wait_clock.add_sem_waits(
        drain_inst.ins, ScopedClock({None: tick_clock.global_clock})
    )
    self.nc.all_engine_barrier()
    popped = self.nc._tile_sem_poison_stack.pop()
    assert popped is self._sem_poison
    self.nc.clear_and_free_semaphores(list(self.sems.allocated().values()))


def _build():
    import concourse.bass as bass
    import concourse.mybir as mybir
    from concourse.bass import ts, ds
    from concourse.tile import TileContext

    TileContext._drain_and_barrier = _cheap_drain_and_barrier

    bf16 = mybir.dt.bfloat16
    f32 = mybir.dt.float32
    GELU = mybir.ActivationFunctionType.Gelu

    nc = bass.Bass("TRN2", target_bir_lowering=False)
    dat_ext = nc.declare_dram_parameter("dat", [SPC, 128, DATW], bf16, isOutput=False)
    bias_ext = nc.declare_dram_parameter("bias", [128, BIASW], f32, isOutput=False)
    out_ext = nc.declare_dram_parameter("out", [SPC, OUT, T], bf16, isOutput=True)

    # HAM warmup: the PE clock gate defaults to 4/8 (1.2GHz) and needs ~3.4us
    # of matmul activity to release. The PE is idle between the entry barrier
    # (~6.5us) and sample 0's data (~10us) — spend it on garbage matmuls over
    # uninitialized SBUF so the real stream starts at 2.4GHz. The PSUM result
    # is dead: these run before the tile body, and every real accumulation
    # group opens with start=True.
    warm_sb = nc.alloc_sbuf_tensor("warm_sb", [128, T], bf16)
    with nc.psum_tensor("warm_ps", [128, T], f32) as warm_ps:
        for _ in range(8):
            nc.tensor.matmul(
                warm_ps.ap()[:],
                warm_sb.ap()[:, 0:128],
                warm_sb.ap()[:],
                start=True,
                stop=True,
            )

    with TileContext(nc) as tc:
        with (
            tc.tile_pool(name="datp", bufs=4) as datp,
            tc.tile_pool(name="bp", bufs=1) as bp,
            tc.tile_pool(name="htp", bufs=3) as htp,
            tc.tile_pool(name="outp", bufs=3) as outp,
            tc.tile_pool(name="ps1", bufs=6, space="PSUM") as ps1p,
            tc.tile_pool(name="ps2", bufs=2, space="PSUM") as ps2p,
        ):
            bt = bp.tile([128, BIASW], f32, name="bt")
            nc.gpsimd.dma_start(out=bt[:], in_=bias_ext[:])

            def fc1(j, prev_datt):
                datt = datp.tile([128, DATW], bf16, name="datt", tag="datt")
                # w2 of the previous sample is first needed by fc2(j-1), which
                # runs right after this fc1 — issue it ahead of our own load
                if prev_datt is not None:
                    nc.sync.dma_start(
                        out=prev_datt[:, W2OFF:], in_=dat_ext[j - 1, :, W2OFF:]
                    )
                if j < 2:
                    # ramp: w1 is m-major, so fc1 m-group 0/1 can start once
                    # xt + the first half-MB of w1 land; rest streams behind
                    s1 = W1OFF + 2 * KT1 * 128
                    nc.sync.dma_start(out=datt[:, :s1], in_=dat_ext[j, :, :s1])
                    nc.sync.dma_start(out=datt[:, s1:W2OFF], in_=dat_ext[j, :, s1:W2OFF])
                else:
                    nc.sync.dma_start(out=datt[:, :W2OFF], in_=dat_ext[j, :, :W2OFF])
                if j == SPC - 1:
                    nc.sync.dma_start(out=datt[:, W2OFF:], in_=dat_ext[j, :, W2OFF:])
                htt = htp.tile([128, KT2, T], bf16, name="htt", tag="htt")
                for m in range(MT1):
                    ps = ps1p.tile([128, T], f32, name="ps", tag="ps")
                    for k in range(KT1):
                        nc.tensor.matmul(
                            ps[:],
                            datt[:, ds(W1OFF + m * KT1 * 128 + k * 128, 128)],
                            datt[:, ds(XTOFF + k * T, T)],
                            start=(k == 0),
                            stop=(k == KT1 - 1),
                        )
                    c = j * MT1 + m
                    nc.scalar.activation(htt[:, m, :], ps[:], GELU, bias=bt[:, c : c + 1])
                return datt, htt

            def fc2(j, datt, htt):
                ott = outp.tile([128, MT2, T], bf16, name="ott", tag="ott")
                for m in range(MT2):
                    ps2 = ps2p.tile([128, T], f32, name="ps2", tag="ps2")
                    for k in range(KT2):
                        nc.tensor.matmul(
                            ps2[:],
                            datt[:, ds(W2OFF + k * OUT + m * 128, 128)],
                            htt[:, k, :],
                            start=(k == 0),
                            stop=(k == KT2 - 1),
                        )
                    c = B2COL + j * MT2 + m
                    nc.vector.tensor_scalar_add(ott[:, m, :], ps2[:], bt[:, c : c + 1])
                    # last sample: spread the final store over two DGE queues
                    # (GpSimd SWDGE + ACT HWDGE) so issue and flight don't
                    # serialize on the kernel-tail critical path
                    if j == SPC - 1 and m == MT2 - 1:
                        half = T // 2
                        nc.sync.dma_start(
                            out=out_ext[j, ts(m, 128), :half], in_=ott[:, m, :half]
                        )
                        nc.scalar.dma_start(
                            out=out_ext[j, ts(m, 128), half:], in_=ott[:, m, half:]
                        )
                    else:
                        nc.gpsimd.dma_start(
                            out=out_ext[j, ts(m, 128), :], in_=ott[:, m, :]
                        )

            # software pipeline: fc2 of sample j-1 runs while fc1 of sample j
            # streams, so PE never stalls on the gelu at the fc1->fc2 boundary
            prev = None
            for j in range(SPC):
                cur = fc1(j, prev[0] if prev else None)
                if prev is not None:
                    fc2(j - 1, *prev)
                prev = cur
            fc2(SPC - 1, *prev)

    _split_multi_waits(nc)
    _hoist_first_dmas(nc)
    return nc


def _hoist_first_dmas(nc, n=2):
    """Move sample 0's first (wait-free) SP DMACopy instructions from the
    tile block into the main block, ahead of the entry all-engine barrier.
    Their data flight then overlaps the Tile preamble (~2.5us earlier PE
    start). Sem increments firing earlier is safe: consumers' wait_ge
    thresholds are absolute."""
    f = nc.m.functions[0]
    main_bb = f.blocks[0]
    tile_bbs = [b for b in f.blocks if "tile_context" in (b.name or "")]
    if not tile_bbs:
        return
    tile_bb = tile_bbs[0]

    hoisted = []
    for inst in list(tile_bb.instructions):
        if len(hoisted) >= n:
            break
        if type(inst).__name__ != "InstDMACopy":
            continue
        if str(inst.engine) != "EngineType.SP":
            continue
        si = inst.sync_info
        if si and si.on_wait:
            break  # first DMA with a wait ends the safely-hoistable prefix
        hoisted.append(inst)
    if not hoisted:
        return
    for inst in hoisted:
        tile_bb.instructions.remove(inst)
    # insert at the very top of main: ahead of the SP reg-setup movs and the
    # entry all-engine barrier (HWDGE descriptor generation reads only the
    # instruction's APs, not the bcreg state those movs initialize)
    main_bb.instructions[0:0] = hoisted


def _run(in_maps, trace=False, **kw):
    from concourse.bass_utils import run_bass_kernel_spmd

    if "nc" not in _CACHE:
        _CACHE["nc"] = _build()
    return run_bass_kernel_spmd(
        _CACHE["nc"], in_maps, list(range(N_CORES)), trace=trace, **kw
    )


def _prep_in_maps(x, hetero_info, fc1_table, fc2_table):
    x = np.asarray(x, dtype=np.float32)
    hetero_info = np.asarray(hetero_info)
    fc1_table = np.asarray(fc1_table, dtype=np.float32)
    fc2_table = np.asarray(fc2_table, dtype=np.float32)
    bf16 = ml_dtypes.bfloat16

    domain = hetero_info[:, 0].astype(np.int64)
    fc1p = fc1_table[domain]  # (B, IN*HID + HID)
    w1 = fc1p[:, : IN * HID].reshape(B, IN, HID).astype(bf16)
    b1 = fc1p[:, IN * HID :]  # (B, HID) f32
    fc2p = fc2_table[domain]
    w2 = fc2p[:, : HID * OUT].reshape(B, HID, OUT).astype(bf16)
    b2 = fc2p[:, HID * OUT :]  # (B, OUT) f32

    xt = np.ascontiguousarray(x.transpose(0, 2, 1)).astype(bf16)  # (B, IN, T)

    # pack per-sample [w1 | w2 | xt] partition-major: data[b, p, :] holds the
    # k-major per-partition rows each matmul slices directly out of SBUF
    # w1 m-major per partition: word W1OFF + m*KT1*128 + k*128 + c
    w1v = (
        w1.reshape(B, KT1, 128, MT1, 128).transpose(0, 2, 3, 1, 4).reshape(B, 128, W1W)
    )
    w2v = w2.reshape(B, KT2, 128, OUT).transpose(0, 2, 1, 3).reshape(B, 128, W2W)
    xtv = xt.reshape(B, KT1, 128, T).transpose(0, 2, 1, 3).reshape(B, 128, XTW)
    dat = np.concatenate([xtv, w1v, w2v], axis=2)  # (B, 128, DATW)

    # biases partition-major: [128, SPC*MT1 | SPC*MT2] per core
    b1t = b1.reshape(N_CORES, SPC * MT1, 128).transpose(0, 2, 1)
    b2t = b2.reshape(N_CORES, SPC * MT2, 128).transpose(0, 2, 1)
    bias = np.concatenate([b1t, b2t], axis=2).astype(np.float32)  # (8, 128, BIASW)

    in_maps = []
    for s in range(N_CORES):
        sl = slice(s * SPC, (s + 1) * SPC)
        in_maps.append(
            {
                "dat": np.ascontiguousarray(dat[sl]),
                "bias": np.ascontiguousarray(bias[s]),
            }
        )
    return in_maps


def _assemble(results):
    outT = np.stack([results[s]["out"] for s in range(N_CORES)])  # (8, SPC, OUT, T)
    return np.ascontiguousarray(
        outT.reshape(B, OUT, T).transpose(0, 2, 1).astype(np.float32)
    )  # (B, T, OUT) f32


def kernel(x, hetero_info, fc1_table, fc2_table):
    import os

    in_maps = _prep_in_maps(x, hetero_info, fc1_table, fc2_table)
    # profiling needs an artifact bucket this container doesn't have; make
    # sure a stray BASS_TRACE in the environment can't pull that path in
    prev = os.environ.get("BASS_NEVER_TRACE")
    os.environ["BASS_NEVER_TRACE"] = "1"
    try:
        res = _run(in_maps, trace=False)
    finally:
        if prev is None:
            os.environ.pop("BASS_NEVER_TRACE", None)
        else:
            os.environ["BASS_NEVER_TRACE"] = prev
    return _assemble(res.results)


# revision 42
# speedup vs baseline: 1.0125x; 1.0125x over previous
"""Trainium2 Bass kernel for nn_ActionLearner (per-sample-expert dense MLP).

reference:
    w1,b1 = fc1_table[domain_id]   # per-sample (512,1024) + (1024,)
    w2,b2 = fc2_table[domain_id]   # per-sample (1024,256) + (256,)
    out = gelu(x @ w1 + b1) @ w2 + b2          # x: (64, 256, 512)

Sharding: data-parallel over batch across 8 NeuronCores (8 samples/core).
The embedding-table gather runs on host (per-sample weights stay local to
each core's batch shard); x is host-transposed to (IN, T) per sample so
both matmuls run with zero on-device transposes:

    fc1: hT[HID,T]  = accumulate over IN of (w1 as lhsT) x (xT as rhs)
    act: gelu(hT + b1) with b1 as a per-partition bias on ACT
    fc2: oT[OUT,T]  = accumulate over HID of (w2 as lhsT) x (hT as rhs)
    out: oT DMA'd out bf16, host transposes back to (T, OUT) f32

Matmul operands are bf16 (f32 PSUM accumulation); biases stay f32.
Each sample's w1|w2|xT are host-packed partition-major into one tensor so
the whole sample loads with a single large DMA (HWDGE issue is ~0.7us per
dma_start on the SP sequencer — fewer, bigger transfers). Bias load and
output stores ride SWDGE on the otherwise-idle GpSimd engine.
"""

import numpy as np
import ml_dtypes

B, T = 64, 256
IN, HID, OUT = 512, 1024, 256
N_CORES = 8
SPC = B // N_CORES  # samples per core
KT1 = IN // 128     # fc1 contraction tiles
MT1 = HID // 128    # fc1 output-partition tiles
KT2 = HID // 128    # fc2 contraction tiles
MT2 = OUT // 128    # fc2 output-partition tiles

W1W = KT1 * HID           # 4096 bf16 words per partition
W2W = KT2 * OUT           # 2048
XTW = KT1 * T             # 1024
DATW = W1W + W2W + XTW    # 7168
XTOFF = 0                 # packed order [xt | w1 | w2]
W1OFF = XTW
W2OFF = XTW + W1W
BIASW = SPC * (MT1 + MT2)  # 80 f32 words per partition
B2COL = SPC * MT1

_CACHE = {}


def _split_multi_waits(nc):
    """This container's walrus build accepts at most ONE sync-wait per
    instruction. Hoist all but the last wait of each instruction onto fresh
    same-engine nops inserted immediately before it — identical semantics,
    engine queues execute in block order."""
    import concourse.mybir as mybir

    f = nc.m.functions[0]
    for bb in f.blocks:
        insts = bb.instructions
        if not any(
            i.sync_info and i.sync_info.on_wait and len(i.sync_info.on_wait) > 1
            for i in insts
        ):
            continue
        new_list = []
        for inst in list(insts):
            si = inst.sync_info
            if si and si.on_wait and len(si.on_wait) > 1:
                extra, keep = si.on_wait[:-1], si.on_wait[-1:]
                si.on_wait = keep
                for w in extra:
                    nop = nc.engines[inst.engine].nop(nofuse=True).ins
                    for b2 in f.blocks:
                        if b2.instructions and b2.instructions[-1] is nop:
                            b2.instructions.pop()
                            break
                    nop.sync_info = mybir.SyncInfo(on_wait=[w], on_update=[])
                    new_list.append(nop)
            new_list.append(inst)
        insts[:] = new_list


def _cheap_drain_and_barrier(self, tick_clock, wait_clock):
    """TileContext exit for a kernel where the context is the last thing in
    the program: drain + one barrier + sem clears, skipping the trailing
    all-engine barrier (nothing runs after the clears; engines just halt)."""
    from concourse.vector_clock import ScopedClock

    drain_inst = self.nc.sync.drain()
    wait_clock.add_sem_waits(
        drain_inst.ins, ScopedClock({None: tick_clock.global_clock})
    )
    self.nc.all_engine_barrier()
    popped = self.nc._tile_sem_poison_stack.pop()
    assert popped is self._sem_poison
    self.nc.clear_and_free_semaphores(list(self.sems.allocated().values()))


def _build():
    import concourse.bass as bass
    import concourse.mybir as mybir
    from concourse.bass import ts, ds
    from concourse.tile import TileContext

    TileContext._drain_and_barrier = _cheap_drain_and_barrier

    bf16 = mybir.dt.bfloat16
    f32 = mybir.dt.float32
    GELU = mybir.ActivationFunctionType.Gelu

    nc = bass.Bass("TRN2", target_bir_lowering=False)
    dat_ext = nc.declare_dram_parameter("dat", [SPC, 128, DATW], bf16, isOutput=False)
    bias_ext = nc.declare_dram_parameter("bias", [128, BIASW], f32, isOutput=False)
    out_ext = nc.declare_dram_parameter("out", [SPC, OUT, T], bf16, isOutput=True)

    # HAM warmup: the PE clock gate defaults to 4/8 (1.2GHz) and needs ~3.4us
    # of matmul activity to release. The PE is idle between the entry barrier
    # (~6.5us) and sample 0's data (~10us) — spend it on garbage matmuls over
    # uninitialized SBUF so the real stream starts at 2.4GHz. The PSUM result
    # is dead: these run before the tile body, and every real accumulation
    # group opens with start=True.
    warm_sb = nc.alloc_sbuf_tensor("warm_sb", [128, T], bf16)
    with nc.psum_tensor("warm_ps", [128, T], f32) as warm_ps:
        for _ in range(8):
            nc.tensor.matmul(
                warm_ps.ap()[:],
                warm_sb.ap()[:, 0:128],
                warm_sb.ap()[:],
                start=True,
                stop=True,
            )

    with TileContext(nc) as tc:
        with (
            tc.tile_pool(name="datp", bufs=4) as datp,
            tc.tile_pool(name="bp", bufs=1) as bp,
            tc.tile_pool(name="htp", bufs=3) as htp,
            tc.tile_pool(name="outp", bufs=3) as outp,
            tc.tile_pool(name="ps1", bufs=6, space="PSUM") as ps1p,
            tc.tile_pool(name="ps2", bufs=2, space="PSUM") as ps2p,
        ):
            bt = bp.tile([128, BIASW], f32, name="bt")
            nc.gpsimd.dma_start(out=bt[:], in_=bias_ext[:])

            def fc1(j, prev_datt):
                datt = datp.tile([128, DATW], bf16, name="datt", tag="datt")
                # w2 of the previous sample feeds fc2(j-1), which runs right
                # after this fc1 — issue it ahead of our own load. Exception:
                # j==1 sits in the DMA-bound ramp where PE consumes A(1)
                # (fc1) before w2(0) (fc2), so there delivery follows
                # consumption order instead.
                if prev_datt is not None and j != 1:
                    nc.sync.dma_start(
                        out=prev_datt[:, W2OFF:], in_=dat_ext[j - 1, :, W2OFF:]
                    )
                if j < 2:
                    # ramp: w1 is m-major, so fc1 m-group 0/1 can start once
                    # xt + the first half-MB of w1 land; rest streams behind
                    s1 = W1OFF + 2 * KT1 * 128
                    nc.sync.dma_start(out=datt[:, :s1], in_=dat_ext[j, :, :s1])
                    nc.sync.dma_start(out=datt[:, s1:W2OFF], in_=dat_ext[j, :, s1:W2OFF])
                else:
                    nc.sync.dma_start(out=datt[:, :W2OFF], in_=dat_ext[j, :, :W2OFF])
                if j == 1:
                    nc.sync.dma_start(
                        out=prev_datt[:, W2OFF:], in_=dat_ext[0, :, W2OFF:]
                    )
                if j == SPC - 1:
                    nc.sync.dma_start(out=datt[:, W2OFF:], in_=dat_ext[j, :, W2OFF:])
                htt = htp.tile([128, KT2, T], bf16, name="htt", tag="htt")
                for m in range(MT1):
                    ps = ps1p.tile([128, T], f32, name="ps", tag="ps")
                    for k in range(KT1):
                        nc.tensor.matmul(
                            ps[:],
                            datt[:, ds(W1OFF + m * KT1 * 128 + k * 128, 128)],
                            datt[:, ds(XTOFF + k * T, T)],
                            start=(k == 0),
                            stop=(k == KT1 - 1),
                        )
                    c = j * MT1 + m
                    nc.scalar.activation(htt[:, m, :], ps[:], GELU, bias=bt[:, c : c + 1])
                return datt, htt

            def fc2(j, datt, htt):
                ott = outp.tile([128, MT2, T], bf16, name="ott", tag="ott")
                for m in range(MT2):
                    ps2 = ps2p.tile([128, T], f32, name="ps2", tag="ps2")
                    for k in range(KT2):
                        nc.tensor.matmul(
                            ps2[:],
                            datt[:, ds(W2OFF + k * OUT + m * 128, 128)],
                            htt[:, k, :],
                            start=(k == 0),
                            stop=(k == KT2 - 1),
                        )
                    c = B2COL + j * MT2 + m
                    nc.vector.tensor_scalar_add(ott[:, m, :], ps2[:], bt[:, c : c + 1])
                    # last sample: spread the final store over two DGE queues
                    # (GpSimd SWDGE + ACT HWDGE) so issue and flight don't
                    # serialize on the kernel-tail critical path
                    if j == SPC - 1 and m == MT2 - 1:
                        half = T // 2
                        nc.sync.dma_start(
                            out=out_ext[j, ts(m, 128), :half], in_=ott[:, m, :half]
                        )
                        nc.scalar.dma_start(
                            out=out_ext[j, ts(m, 128), half:], in_=ott[:, m, half:]
                        )
                    else:
                        nc.gpsimd.dma_start(
                            out=out_ext[j, ts(m, 128), :], in_=ott[:, m, :]
                        )

            # software pipeline: fc2 of sample j-1 runs while fc1 of sample j
            # streams, so PE never stalls on the gelu at the fc1->fc2 boundary
            prev = None
            for j in range(SPC):
                cur = fc1(j, prev[0] if prev else None)
                if prev is not None:
                    fc2(j - 1, *prev)
                prev = cur
            fc2(SPC - 1, *prev)

    _split_multi_waits(nc)
    _hoist_first_dmas(nc)
    return nc


def _hoist_first_dmas(nc, n=2):
    """Move sample 0's first (wait-free) SP DMACopy instructions from the
    tile block into the main block, ahead of the entry all-engine barrier.
    Their data flight then overlaps the Tile preamble (~2.5us earlier PE
    start). Sem increments firing earlier is safe: consumers' wait_ge
    thresholds are absolute."""
    f = nc.m.functions[0]
    main_bb = f.blocks[0]
    tile_bbs = [b for b in f.blocks if "tile_context" in (b.name or "")]
    if not tile_bbs:
        return
    tile_bb = tile_bbs[0]

    hoisted = []
    for inst in list(tile_bb.instructions):
        if len(hoisted) >= n:
            break
        if type(inst).__name__ != "InstDMACopy":
            continue
        if str(inst.engine) != "EngineType.SP":
            continue
        si = inst.sync_info
        if si and si.on_wait:
            break  # first DMA with a wait ends the safely-hoistable prefix
        hoisted.append(inst)
    if not hoisted:
        return
    for inst in hoisted:
        tile_bb.instructions.remove(inst)
    # insert at the very top of main: ahead of the SP reg-setup movs and the
    # entry all-engine barrier (HWDGE descriptor generation reads only the
    # instruction's APs, not the bcreg state those movs initialize)
    main_bb.instructions[0:0] = hoisted


def _run(in_maps, trace=False, **kw):
    from concourse.bass_utils import run_bass_kernel_spmd

    if "nc" not in _CACHE:
        _CACHE["nc"] = _build()
    return run_bass_kernel_spmd(
        _CACHE["nc"], in_maps, list(range(N_CORES)), trace=trace, **kw
    )


def _prep_in_maps(x, hetero_info, fc1_table, fc2_table):
    x = np.asarray(x, dtype=np.float32)
    hetero_info = np.asarray(hetero_info)
    fc1_table = np.asarray(fc1_table, dtype=np.float32)
    fc2_table = np.asarray(fc2_table, dtype=np.float32)
    bf16 = ml_dtypes.bfloat16

    domain = hetero_info[:, 0].astype(np.int64)
    fc1p = fc1_table[domain]  # (B, IN*HID + HID)
    w1 = fc1p[:, : IN * HID].reshape(B, IN, HID).astype(bf16)
    b1 = fc1p[:, IN * HID :]  # (B, HID) f32
    fc2p = fc2_table[domain]
    w2 = fc2p[:, : HID * OUT].reshape(B, HID, OUT).astype(bf16)
    b2 = fc2p[:, HID * OUT :]  # (B, OUT) f32

    xt = np.ascontiguousarray(x.transpose(0, 2, 1)).astype(bf16)  # (B, IN, T)

    # pack per-sample [w1 | w2 | xt] partition-major: data[b, p, :] holds the
    # k-major per-partition rows each matmul slices directly out of SBUF
    # w1 m-major per partition: word W1OFF + m*KT1*128 + k*128 + c
    w1v = (
        w1.reshape(B, KT1, 128, MT1, 128).transpose(0, 2, 3, 1, 4).reshape(B, 128, W1W)
    )
    w2v = w2.reshape(B, KT2, 128, OUT).transpose(0, 2, 1, 3).reshape(B, 128, W2W)
    xtv = xt.reshape(B, KT1, 128, T).transpose(0, 2, 1, 3).reshape(B, 128, XTW)
    dat = np.concatenate([xtv, w1v, w2v], axis=2)  # (B, 128, DATW)

    # biases partition-major: [128, SPC*MT1 | SPC*MT2] per core
    b1t = b1.reshape(N_CORES, SPC * MT1, 128).transpose(0, 2, 1)
    b2t = b2.reshape(N_CORES, SPC * MT2, 128).transpose(0, 2, 1)
    bias = np.concatenate([b1t, b2t], axis=2).astype(np.float32)  # (8, 128, BIASW)

    in_maps = []
    for s in range(N_CORES):
        sl = slice(s * SPC, (s + 1) * SPC)
        in_maps.append(
            {
                "dat": np.ascontiguousarray(dat[sl]),
                "bias": np.ascontiguousarray(bias[s]),
            }
        )
    return in_maps


def _assemble(results):
    outT = np.stack([results[s]["out"] for s in range(N_CORES)])  # (8, SPC, OUT, T)
    return np.ascontiguousarray(
        outT.reshape(B, OUT, T).transpose(0, 2, 1).astype(np.float32)
    )  # (B, T, OUT) f32


def kernel(x, hetero_info, fc1_table, fc2_table):
    import os

    in_maps = _prep_in_maps(x, hetero_info, fc1_table, fc2_table)
    # profiling needs an artifact bucket this container doesn't have; make
    # sure a stray BASS_TRACE in the environment can't pull that path in
    prev = os.environ.get("BASS_NEVER_TRACE")
    os.environ["BASS_NEVER_TRACE"] = "1"
    try:
        res = _run(in_maps, trace=False)
    finally:
        if prev is None:
            os.environ.pop("BASS_NEVER_TRACE", None)
        else:
            os.environ["BASS_NEVER_TRACE"] = prev
    return _assemble(res.results)
